# revision 2
# baseline (speedup 1.0000x reference)
"""DiceLoss partial-sum kernel for Trainium2 (8 NeuronCores, data-parallel).

Computes, for input/target of shape (32, 1, 1024, 1024) fp32:
    bin   = (input > 0.5) ? 1.0 : 0.0
    loss1 = 2 * sum(bin * target)
    loss2 = sum(bin) + sum(target)
and returns (loss1, loss2) as fp32 scalars (same structure as the reference).

Sharding: batch dim N=32 is split 4-per-core across 8 cores. Each core
streams its 16 MiB input + 16 MiB target shard through SBUF as [128, F]
fp32 tiles via HWDGE DMA. The problem is HBM-bound (~80 us of DMA per core
at the ~425 GB/s fair share of the chip's aggregate bandwidth; strided
high-byte reads don't help - the DMA ISA lowers non-contiguous innermost
dims to per-element descriptors).

v2 design (deep-runway): the previous 4-slot ring paced late-tile DMAs on
compute completion, which on contended runs collapsed the last ~2 MiB into
a 20+ us latency-bound convoy (queues drained to 1-2 small tiles in
flight). Now each tensor streams through a 20480-elem/partition SBUF arena
(20 MiB total runway): the first 5 x 4096-wide tile pairs are DMA'd with
no waits at all, later tiles only wait on consumers of the early tiles
whose arena range they reuse (satisfied long before the queues drain), so
the DMA queues stay descriptor-fed for the whole 32 MiB and the stream
runs at the HBM share until the last byte. The tile sizes taper
(...4096, 2048, 1024, 512, 512) so the compute tail after the last byte
lands is ~1 us.

Engines: per tile, vector does STT (in>0.5)*tgt accum -> loss1 column;
"dve" tiles also STT (in>0.5)+tgt accum -> loss2 column (exact). "act"
tiles instead use scalar: Copy(tgt) accum -> tgt column and Sign(1-2*in)
accum -> sign column (bin count recovered on host as (n - S')/2). Split
balances vector/scalar at ~44 us each, both well under the ~80 us DMA
window. Dummy STT/ACT outputs are written to small bf16 scratch (the
accumulator taps are what we keep), freeing SBUF for the arena. gpsimd
zeroes the stats tile once at start. Per-partition partial sums land in a
[128, 3*nt] stats tile DMA'd out per core; the final tiny reduction over
cores/partitions/tiles happens on the host in float64.
"""

from contextlib import ExitStack

import numpy as np

try:
    import concourse.bass  # noqa: F401
except ImportError:  # pragma: no cover - path fallback for bare containers
    import sys

    for _p in ("/opt/trn_rl_repo", "/root/.axon_site/_ro/trn_rl_repo"):
        if _p not in sys.path:
            sys.path.insert(0, _p)

import concourse.bacc as bacc
import concourse.mybir as mybir
from concourse.bass_utils import run_bass_kernel_spmd

N_CORES = 8
FULL_SHAPE = (32, 1, 1024, 1024)
FULL_ELEMS = 32 * 1024 * 1024
PER_CORE = FULL_ELEMS // N_CORES  # 4_194_304
P = 128
FREE = PER_CORE // P  # 32768 fp32 elements per partition per tensor
THRESH = 0.5
ARENA = 20480  # fp32 elems/partition per tensor arena (80 KiB); 5 x 4096

# (free_size, loss2_mode); sizes sum to FREE. Tiles are placed at
# cumulative-offset mod ARENA; each fits a 4096-aligned lane so no tile
# straddles the wrap.
TILES = tuple(
    [(4096, "act"), (4096, "act"), (4096, "dve"), (4096, "act"),
     (4096, "act"), (4096, "dve"), (4096, "act"),
     (2048, "act"), (1024, "act"), (512, "dve"), (512, "act")]
)
assert sum(f for f, _ in TILES) == FREE

_CACHE: dict = {}


def _layout(tiles):
    """Arena offsets + (first tile whose consumers must finish before
    this tile's DMA may overwrite its arena range, or None)."""
    offs_dram, offs_arena, waits = [], [], []
    live = {}  # arena_off -> tile idx currently owning [off, off+tf)
    off = 0
    for t, (tf, _) in enumerate(tiles):
        offs_dram.append(off)
        a = off % ARENA
        assert a + tf <= ARENA
        offs_arena.append(a)
        # which earlier tile(s) overlap [a, a+tf)? tiles are 4096-lane
        # aligned sub-ranges, so checking the lane owner is enough
        lane = a - (a % 4096)
        waits.append(live.get(lane))
        live[lane] = t
        off += tf
    return offs_dram, offs_arena, waits


def _build(tiles: tuple, n_cores: int):
    f32 = mybir.dt.float32
    bf16 = mybir.dt.bfloat16
    nt = len(tiles)
    per_core = P * sum(f for f, _ in tiles)
    max_f = max(f for f, _ in tiles)
    nc = bacc.Bacc(
        "TRN2", target_bir_lowering=False, debug=False, num_devices=n_cores
    )
    inp = nc.dram_tensor("input", [per_core], f32, kind="ExternalInput").ap()
    tgt = nc.dram_tensor("target", [per_core], f32, kind="ExternalInput").ap()
    stats = nc.dram_tensor("stats", [P, 3 * nt], f32, kind="ExternalOutput").ap()

    ti_ar = nc.alloc_sbuf_tensor("ti_ar", [P, ARENA], f32).ap()
    tt_ar = nc.alloc_sbuf_tensor("tt_ar", [P, ARENA], f32).ap()
    # dummy compute outputs (accumulator taps are the real product);
    # two per engine, alternated so consecutive same-engine instructions
    # never write the same buffer (deep-pipeline WAW)
    sd = [nc.alloc_sbuf_tensor(f"sd{i}", [P, max_f], bf16).ap() for i in range(2)]
    sa = [nc.alloc_sbuf_tensor(f"sa{i}", [P, max_f], bf16).ap() for i in range(2)]
    st = nc.alloc_sbuf_tensor("st", [P, 3 * nt], f32).ap()

    offs_dram, offs_arena, waits = _layout(tiles)
    V = []  # cumulative vector instr count through tile t
    S = []  # cumulative scalar instr count through tile t
    v = s = 0
    for tf, mode in tiles:
        v += 2 if mode == "dve" else 1
        s += 0 if mode == "dve" else 2
        V.append(v)
        S.append(s)

    with ExitStack() as ctx:
        tile_sems = [
            ctx.enter_context(nc.semaphore(f"tile_sem{i}")) for i in range(nt)
        ]
        vec_sem = ctx.enter_context(nc.semaphore("vec_sem"))
        sc_sem = ctx.enter_context(nc.semaphore("sc_sem"))
        gp_sem = ctx.enter_context(nc.semaphore("gp_sem"))
        out_sem = ctx.enter_context(nc.semaphore("out_sem"))
        block = ctx.enter_context(nc.Block())

        @block.gpsimd
        def _(gpsimd):
            gpsimd.memset(st[:], 0.0).then_inc(gp_sem, 1)

        @block.sync
        def _(sync):
            wv = ws = 0  # highest consumer counts already waited on
            for t, (tf, mode) in enumerate(tiles):
                w = waits[t]
                if w is not None:
                    if V[w] > wv:
                        sync.wait_ge(vec_sem, V[w])
                        wv = V[w]
                    if S[w] > ws:
                        sync.wait_ge(sc_sem, S[w])
                        ws = S[w]
                o, a = offs_dram[t], offs_arena[t]
                src_i = inp[o * P : (o + tf) * P].rearrange("(p f) -> p f", p=P)
                src_t = tgt[o * P : (o + tf) * P].rearrange("(p f) -> p f", p=P)
                sem = tile_sems[t]
                sync.dma_start(out=ti_ar[:, a : a + tf], in_=src_i).then_inc(
                    sem, 16
                )
                sync.dma_start(out=tt_ar[:, a : a + tf], in_=src_t).then_inc(
                    sem, 16
                )
            # sem update on an accum instruction fires at full instruction
            # retirement (incl. the accumulator write-back), so the stats DMA
            # can depend on the compute sems directly - no fence instructions
            sync.wait_ge(vec_sem, V[-1])
            sync.wait_ge(sc_sem, S[-1])
            sync.wait_ge(gp_sem, 1)
            sync.dma_start(out=stats[:], in_=st[:]).then_inc(out_sem, 16)
            sync.wait_ge(out_sem, 16)

        @block.vector
        def _(vector):
            vector.wait_ge(gp_sem, 1)
            vi = 0
            for t, (tf, mode) in enumerate(tiles):
                a = offs_arena[t]
                vector.wait_ge(tile_sems[t], 32)
                if vi >= 2:
                    # scratch-reuse self-wait; satisfied by in-order retirement
                    vector.wait_ge(vec_sem, vi - 1)
                vector.scalar_tensor_tensor(
                    out=sd[vi % 2][:, :tf],
                    in0=ti_ar[:, a : a + tf],
                    scalar=THRESH,
                    in1=tt_ar[:, a : a + tf],
                    op0=mybir.AluOpType.is_gt,
                    op1=mybir.AluOpType.mult,
                    accum_out=st[:, t : t + 1],
                ).then_inc(vec_sem, 1)
                vi += 1
                if mode == "dve":
                    if vi >= 2:
                        vector.wait_ge(vec_sem, vi - 1)
                    vector.scalar_tensor_tensor(
                        out=sd[vi % 2][:, :tf],
                        in0=ti_ar[:, a : a + tf],
                        scalar=THRESH,
                        in1=tt_ar[:, a : a + tf],
                        op0=mybir.AluOpType.is_gt,
                        op1=mybir.AluOpType.add,
                        accum_out=st[:, nt + t : nt + t + 1],
                    ).then_inc(vec_sem, 1)
                    vi += 1

        @block.scalar
        def _(scalar):
            scalar.wait_ge(gp_sem, 1)
            si = 0
            for t, (tf, mode) in enumerate(tiles):
                if mode == "dve":
                    continue
                a = offs_arena[t]
                scalar.wait_ge(tile_sems[t], 32)
                if si >= 2:
                    scalar.wait_ge(sc_sem, si - 1)
                scalar.activation(
                    out=sa[0][:, :tf],
                    in_=tt_ar[:, a : a + tf],
                    func=mybir.ActivationFunctionType.Copy,
                    accum_out=st[:, nt + t : nt + t + 1],
                ).then_inc(sc_sem, 1)
                si += 1
                if si >= 2:
                    scalar.wait_ge(sc_sem, si - 1)
                # Sign(1 - 2x) = -Sign(x - 0.5); bias=1.0 has a pre-registered
                # const AP, the host negates
                scalar.activation(
                    out=sa[1][:, :tf],
                    in_=ti_ar[:, a : a + tf],
                    func=mybir.ActivationFunctionType.Sign,
                    bias=1.0,
                    scale=-2.0,
                    accum_out=st[:, 2 * nt + t : 2 * nt + t + 1],
                ).then_inc(sc_sem, 1)
                si += 1

    nc.compile()
    return nc


def _get_nc():
    key = (TILES, N_CORES)
    if key not in _CACHE:
        _CACHE[key] = _build(*key)
    return _CACHE[key]


def kernel(input: np.ndarray, target: np.ndarray, **run_kwargs):
    inp = np.asarray(input, dtype=np.float32).reshape(N_CORES, PER_CORE)
    tgt = np.asarray(target, dtype=np.float32).reshape(N_CORES, PER_CORE)

    nc = _get_nc()
    in_maps = [
        {"input": np.ascontiguousarray(inp[c]), "target": np.ascontiguousarray(tgt[c])}
        for c in range(N_CORES)
    ]
    res = run_bass_kernel_spmd(nc, in_maps, core_ids=list(range(N_CORES)), **run_kwargs)

    nt = len(TILES)
    act_tiles = [t for t, (_, m) in enumerate(TILES) if m == "act"]
    inter = 0.0
    loss2 = 0.0
    sign_sum = 0.0
    for c in range(N_CORES):
        stats = res.results[c]["stats"].astype(np.float64)
        inter += stats[:, :nt].sum()
        # "dve" tiles: direct (bin + tgt) partials; "act" tiles: Copy -> tgt sums
        loss2 += stats[:, nt : 2 * nt].sum()
        sign_sum += sum(stats[:, 2 * nt + t].sum() for t in act_tiles)
    # "act" tiles' bin count from sign sums: S' = #lt - #gt -> bin = (n - S')/2
    n_act_elems = N_CORES * P * sum(TILES[t][0] for t in act_tiles)
    loss2 += (n_act_elems - sign_sum) / 2.0

    loss1 = np.float32(2.0 * inter)
    loss2 = np.float32(loss2)
    out = (loss1, loss2)
    if run_kwargs.get("trace"):
        return out, res
    return out


# revision 3
# speedup vs baseline: 1.3315x; 1.3315x over previous
"""DiceLoss partial-sum kernel for Trainium2 (8 NeuronCores, data-parallel).

Computes, for input/target of shape (32, 1, 1024, 1024) fp32:
    bin   = (input > 0.5) ? 1.0 : 0.0
    loss1 = 2 * sum(bin * target)
    loss2 = sum(bin) + sum(target)
and returns (loss1, loss2) as fp32 scalars (same structure as the reference).

Sharding: batch dim N=32 is split 4-per-core across 8 cores. Each core
streams its 16 MiB input + 16 MiB target shard through SBUF as [128, F]
fp32 tiles via HWDGE DMA. The problem is HBM-bound (~80 us of DMA per core
at the ~425 GB/s fair share of the chip's aggregate bandwidth; strided
high-byte reads don't help - the DMA ISA lowers non-contiguous innermost
dims to per-element descriptors).

v2 design (deep-runway): the previous 4-slot ring paced late-tile DMAs on
compute completion, which on contended runs collapsed the last ~2 MiB into
a 20+ us latency-bound convoy (queues drained to 1-2 small tiles in
flight). Now each tensor streams through a 20480-elem/partition SBUF arena
(20 MiB total runway): the first 5 x 4096-wide tile pairs are DMA'd with
no waits at all, later tiles only wait on consumers of the early tiles
whose arena range they reuse (satisfied long before the queues drain), so
the DMA queues stay descriptor-fed for the whole 32 MiB and the stream
runs at the HBM share until the last byte. The tile sizes taper
(...4096, 2048, 1024, 512, 512) so the compute tail after the last byte
lands is ~1 us.

Engines: per tile, vector does STT (in>0.5)*tgt accum -> loss1 column;
"dve" tiles also STT (in>0.5)+tgt accum -> loss2 column (exact). "act"
tiles instead use scalar: Copy(tgt) accum -> tgt column and Sign(1-2*in)
accum -> sign column (bin count recovered on host as (n - S')/2). Split
balances vector/scalar at ~44 us each, both well under the ~80 us DMA
window. Dummy STT/ACT outputs are written to small bf16 scratch (the
accumulator taps are what we keep), freeing SBUF for the arena. gpsimd
zeroes the stats tile once at start. Per-partition partial sums land in a
[128, 3*nt] stats tile DMA'd out per core; the final tiny reduction over
cores/partitions/tiles happens on the host in float64.
"""

from contextlib import ExitStack

import numpy as np

try:
    import concourse.bass  # noqa: F401
except ImportError:  # pragma: no cover - path fallback for bare containers
    import sys

    for _p in ("/opt/trn_rl_repo", "/root/.axon_site/_ro/trn_rl_repo"):
        if _p not in sys.path:
            sys.path.insert(0, _p)

import concourse.bacc as bacc
import concourse.mybir as mybir
from concourse.bass_utils import run_bass_kernel_spmd

N_CORES = 8
FULL_SHAPE = (32, 1, 1024, 1024)
FULL_ELEMS = 32 * 1024 * 1024
PER_CORE = FULL_ELEMS // N_CORES  # 4_194_304
P = 128
FREE = PER_CORE // P  # 32768 fp32 elements per partition per tensor
THRESH = 0.5
ARENA = 20480  # fp32 elems/partition per tensor arena (80 KiB); 5 x 4096

# (free_size, loss2_mode); sizes sum to FREE. Tiles are placed at
# cumulative-offset mod ARENA; each fits a 4096-aligned lane so no tile
# straddles the wrap.
TILES = tuple(
    [(4096, "act"), (4096, "act"), (4096, "dve"), (4096, "act"),
     (4096, "act"), (4096, "dve"), (4096, "act"),
     (2048, "act"), (1024, "act"), (512, "dve"), (512, "act")]
)
assert sum(f for f, _ in TILES) == FREE

_CACHE: dict = {}


def _layout(tiles):
    """Arena offsets + (first tile whose consumers must finish before
    this tile's DMA may overwrite its arena range, or None)."""
    offs_dram, offs_arena, waits = [], [], []
    ranges = []  # per tile: (arena_start, arena_end)
    off = 0
    for t, (tf, _) in enumerate(tiles):
        offs_dram.append(off)
        a = off % ARENA
        assert a + tf <= ARENA
        offs_arena.append(a)
        # latest earlier tile whose arena range intersects [a, a+tf)
        w = None
        for u in range(t - 1, -1, -1):
            ua, ue = ranges[u]
            if ua < a + tf and a < ue:
                w = u
                break
        waits.append(w)
        ranges.append((a, a + tf))
        off += tf
    return offs_dram, offs_arena, waits


def _build(tiles: tuple, n_cores: int):
    f32 = mybir.dt.float32
    bf16 = mybir.dt.bfloat16
    nt = len(tiles)
    per_core = P * sum(f for f, _ in tiles)
    max_f = max(f for f, _ in tiles)
    nc = bacc.Bacc(
        "TRN2", target_bir_lowering=False, debug=False, num_devices=n_cores
    )
    inp = nc.dram_tensor("input", [per_core], f32, kind="ExternalInput").ap()
    tgt = nc.dram_tensor("target", [per_core], f32, kind="ExternalInput").ap()
    stats = nc.dram_tensor("stats", [P, 3 * nt], f32, kind="ExternalOutput").ap()

    ti_ar = nc.alloc_sbuf_tensor("ti_ar", [P, ARENA], f32).ap()
    tt_ar = nc.alloc_sbuf_tensor("tt_ar", [P, ARENA], f32).ap()
    # dummy compute outputs (accumulator taps are the real product);
    # two per engine, alternated so consecutive same-engine instructions
    # never write the same buffer (deep-pipeline WAW)
    sd = [nc.alloc_sbuf_tensor(f"sd{i}", [P, max_f], bf16).ap() for i in range(2)]
    sa = [nc.alloc_sbuf_tensor(f"sa{i}", [P, max_f], bf16).ap() for i in range(2)]
    st = nc.alloc_sbuf_tensor("st", [P, 3 * nt], f32).ap()

    offs_dram, offs_arena, waits = _layout(tiles)
    V = []  # cumulative vector instr count through tile t
    S = []  # cumulative scalar instr count through tile t
    v = s = 0
    for tf, mode in tiles:
        v += 2 if mode == "dve" else 1
        s += 0 if mode == "dve" else 2
        V.append(v)
        S.append(s)

    with ExitStack() as ctx:
        tile_sems = [
            ctx.enter_context(nc.semaphore(f"tile_sem{i}")) for i in range(nt)
        ]
        vec_sem = ctx.enter_context(nc.semaphore("vec_sem"))
        sc_sem = ctx.enter_context(nc.semaphore("sc_sem"))
        gp_sem = ctx.enter_context(nc.semaphore("gp_sem"))
        out_sem = ctx.enter_context(nc.semaphore("out_sem"))
        block = ctx.enter_context(nc.Block())

        @block.gpsimd
        def _(gpsimd):
            gpsimd.memset(st[:], 0.0).then_inc(gp_sem, 1)

        @block.sync
        def _(sync):
            wv = ws = 0  # highest consumer counts already waited on
            for t, (tf, mode) in enumerate(tiles):
                w = waits[t]
                if w is not None:
                    if V[w] > wv:
                        sync.wait_ge(vec_sem, V[w])
                        wv = V[w]
                    if S[w] > ws:
                        sync.wait_ge(sc_sem, S[w])
                        ws = S[w]
                o, a = offs_dram[t], offs_arena[t]
                src_i = inp[o * P : (o + tf) * P].rearrange("(p f) -> p f", p=P)
                src_t = tgt[o * P : (o + tf) * P].rearrange("(p f) -> p f", p=P)
                sem = tile_sems[t]
                sync.dma_start(out=ti_ar[:, a : a + tf], in_=src_i).then_inc(
                    sem, 16
                )
                sync.dma_start(out=tt_ar[:, a : a + tf], in_=src_t).then_inc(
                    sem, 16
                )
            # sem update on an accum instruction fires at full instruction
            # retirement (incl. the accumulator write-back), so the stats DMA
            # can depend on the compute sems directly - no fence instructions
            sync.wait_ge(vec_sem, V[-1])
            sync.wait_ge(sc_sem, S[-1])
            sync.wait_ge(gp_sem, 1)
            sync.dma_start(out=stats[:], in_=st[:]).then_inc(out_sem, 16)
            sync.wait_ge(out_sem, 16)

        @block.vector
        def _(vector):
            vector.wait_ge(gp_sem, 1)
            vi = 0
            for t, (tf, mode) in enumerate(tiles):
                a = offs_arena[t]
                vector.wait_ge(tile_sems[t], 32)
                if vi >= 2:
                    # scratch-reuse self-wait; satisfied by in-order retirement
                    vector.wait_ge(vec_sem, vi - 1)
                vector.scalar_tensor_tensor(
                    out=sd[vi % 2][:, :tf],
                    in0=ti_ar[:, a : a + tf],
                    scalar=THRESH,
                    in1=tt_ar[:, a : a + tf],
                    op0=mybir.AluOpType.is_gt,
                    op1=mybir.AluOpType.mult,
                    accum_out=st[:, t : t + 1],
                ).then_inc(vec_sem, 1)
                vi += 1
                if mode == "dve":
                    if vi >= 2:
                        vector.wait_ge(vec_sem, vi - 1)
                    vector.scalar_tensor_tensor(
                        out=sd[vi % 2][:, :tf],
                        in0=ti_ar[:, a : a + tf],
                        scalar=THRESH,
                        in1=tt_ar[:, a : a + tf],
                        op0=mybir.AluOpType.is_gt,
                        op1=mybir.AluOpType.add,
                        accum_out=st[:, nt + t : nt + t + 1],
                    ).then_inc(vec_sem, 1)
                    vi += 1

        @block.scalar
        def _(scalar):
            scalar.wait_ge(gp_sem, 1)
            si = 0
            for t, (tf, mode) in enumerate(tiles):
                if mode == "dve":
                    continue
                a = offs_arena[t]
                scalar.wait_ge(tile_sems[t], 32)
                if si >= 2:
                    scalar.wait_ge(sc_sem, si - 1)
                scalar.activation(
                    out=sa[0][:, :tf],
                    in_=tt_ar[:, a : a + tf],
                    func=mybir.ActivationFunctionType.Copy,
                    accum_out=st[:, nt + t : nt + t + 1],
                ).then_inc(sc_sem, 1)
                si += 1
                if si >= 2:
                    scalar.wait_ge(sc_sem, si - 1)
                # Sign(1 - 2x) = -Sign(x - 0.5); bias=1.0 has a pre-registered
                # const AP, the host negates
                scalar.activation(
                    out=sa[1][:, :tf],
                    in_=ti_ar[:, a : a + tf],
                    func=mybir.ActivationFunctionType.Sign,
                    bias=1.0,
                    scale=-2.0,
                    accum_out=st[:, 2 * nt + t : 2 * nt + t + 1],
                ).then_inc(sc_sem, 1)
                si += 1

    nc.compile()
    return nc


def _get_nc():
    key = (TILES, N_CORES)
    if key not in _CACHE:
        _CACHE[key] = _build(*key)
    return _CACHE[key]


def kernel(input: np.ndarray, target: np.ndarray, **run_kwargs):
    inp = np.asarray(input, dtype=np.float32).reshape(N_CORES, PER_CORE)
    tgt = np.asarray(target, dtype=np.float32).reshape(N_CORES, PER_CORE)

    nc = _get_nc()
    in_maps = [
        {"input": np.ascontiguousarray(inp[c]), "target": np.ascontiguousarray(tgt[c])}
        for c in range(N_CORES)
    ]
    res = run_bass_kernel_spmd(nc, in_maps, core_ids=list(range(N_CORES)), **run_kwargs)

    nt = len(TILES)
    act_tiles = [t for t, (_, m) in enumerate(TILES) if m == "act"]
    inter = 0.0
    loss2 = 0.0
    sign_sum = 0.0
    for c in range(N_CORES):
        stats = res.results[c]["stats"].astype(np.float64)
        inter += stats[:, :nt].sum()
        # "dve" tiles: direct (bin + tgt) partials; "act" tiles: Copy -> tgt sums
        loss2 += stats[:, nt : 2 * nt].sum()
        sign_sum += sum(stats[:, 2 * nt + t].sum() for t in act_tiles)
    # "act" tiles' bin count from sign sums: S' = #lt - #gt -> bin = (n - S')/2
    n_act_elems = N_CORES * P * sum(TILES[t][0] for t in act_tiles)
    loss2 += (n_act_elems - sign_sum) / 2.0

    loss1 = np.float32(2.0 * inter)
    loss2 = np.float32(loss2)
    out = (loss1, loss2)
    if run_kwargs.get("trace"):
        return out, res
    return out


# revision 4
# speedup vs baseline: 1.3714x; 1.0300x over previous
"""DiceLoss partial-sum kernel for Trainium2 (8 NeuronCores, data-parallel).

Computes, for input/target of shape (32, 1, 1024, 1024) fp32:
    bin   = (input > 0.5) ? 1.0 : 0.0
    loss1 = 2 * sum(bin * target)
    loss2 = sum(bin) + sum(target)
and returns (loss1, loss2) as fp32 scalars (same structure as the reference).

Sharding: batch dim N=32 is split 4-per-core across 8 cores. Each core
streams its 16 MiB input + 16 MiB target shard through SBUF as [128, F]
fp32 tiles via HWDGE DMA. The problem is HBM-bound (~80 us of DMA per core
at the ~425 GB/s fair share of the chip's aggregate bandwidth; strided
high-byte reads don't help - the DMA ISA lowers non-contiguous innermost
dims to per-element descriptors).

v2 design (deep-runway): the previous 4-slot ring paced late-tile DMAs on
compute completion, which on contended runs collapsed the last ~2 MiB into
a 20+ us latency-bound convoy (queues drained to 1-2 small tiles in
flight). Now each tensor streams through a 20480-elem/partition SBUF arena
(20 MiB total runway): the first 5 x 4096-wide tile pairs are DMA'd with
no waits at all, later tiles only wait on consumers of the early tiles
whose arena range they reuse (satisfied long before the queues drain), so
the DMA queues stay descriptor-fed for the whole 32 MiB and the stream
runs at the HBM share until the last byte. The tile sizes taper
(...4096, 2048, 1024, 512, 512) so the compute tail after the last byte
lands is ~1 us.

Engines: per tile, vector does STT (in>0.5)*tgt accum -> loss1 column;
"dve" tiles also STT (in>0.5)+tgt accum -> loss2 column (exact). "act"
tiles instead use scalar: Copy(tgt) accum -> tgt column and Sign(1-2*in)
accum -> sign column (bin count recovered on host as (n - S')/2). Split
balances vector/scalar at ~44 us each, both well under the ~80 us DMA
window. Dummy STT/ACT outputs are written to small bf16 scratch (the
accumulator taps are what we keep), freeing SBUF for the arena. gpsimd
zeroes the stats tile once at start. Per-partition partial sums land in a
[128, 3*nt] stats tile DMA'd out per core; the final tiny reduction over
cores/partitions/tiles happens on the host in float64.
"""

from contextlib import ExitStack

import numpy as np

try:
    import concourse.bass  # noqa: F401
except ImportError:  # pragma: no cover - path fallback for bare containers
    import sys

    for _p in ("/opt/trn_rl_repo", "/root/.axon_site/_ro/trn_rl_repo"):
        if _p not in sys.path:
            sys.path.insert(0, _p)

import concourse.bacc as bacc
import concourse.mybir as mybir
from concourse.bass_utils import run_bass_kernel_spmd

N_CORES = 8
FULL_SHAPE = (32, 1, 1024, 1024)
FULL_ELEMS = 32 * 1024 * 1024
PER_CORE = FULL_ELEMS // N_CORES  # 4_194_304
P = 128
FREE = PER_CORE // P  # 32768 fp32 elements per partition per tensor
THRESH = 0.5
ARENA = 20480  # fp32 elems/partition per tensor arena (80 KiB); 5 x 4096

# (free_size, loss2_mode); sizes sum to FREE. Tiles are placed at
# cumulative-offset mod ARENA; no tile straddles the wrap. The taper +
# mode order came from brute-forcing a two-engine in-order timing model
# against the measured 428 GB/s arrival rate: scalar handles loss2 for
# the big early tiles (its 2-pass chain must not extend past the last
# byte), the two "dve" tiles relieve scalar right where its chain would
# otherwise back up, and the final tiny act tiles cost ~0.7 us.
TILES = tuple(
    [(4096, "act"), (4096, "act"), (4096, "act"), (4096, "act"),
     (4096, "act"), (4096, "act"),
     (2048, "act"), (2048, "dve"), (2048, "act"),
     (1024, "dve"), (512, "act"), (256, "act"), (256, "act")]
)
assert sum(f for f, _ in TILES) == FREE

_CACHE: dict = {}


def _layout(tiles):
    """Arena offsets + (first tile whose consumers must finish before
    this tile's DMA may overwrite its arena range, or None)."""
    offs_dram, offs_arena, waits = [], [], []
    ranges = []  # per tile: (arena_start, arena_end)
    off = 0
    for t, (tf, _) in enumerate(tiles):
        offs_dram.append(off)
        a = off % ARENA
        assert a + tf <= ARENA
        offs_arena.append(a)
        # latest earlier tile whose arena range intersects [a, a+tf)
        w = None
        for u in range(t - 1, -1, -1):
            ua, ue = ranges[u]
            if ua < a + tf and a < ue:
                w = u
                break
        waits.append(w)
        ranges.append((a, a + tf))
        off += tf
    return offs_dram, offs_arena, waits


def _build(tiles: tuple, n_cores: int):
    f32 = mybir.dt.float32
    bf16 = mybir.dt.bfloat16
    nt = len(tiles)
    per_core = P * sum(f for f, _ in tiles)
    max_f = max(f for f, _ in tiles)
    nc = bacc.Bacc(
        "TRN2", target_bir_lowering=False, debug=False, num_devices=n_cores
    )
    inp = nc.dram_tensor("input", [per_core], f32, kind="ExternalInput").ap()
    tgt = nc.dram_tensor("target", [per_core], f32, kind="ExternalInput").ap()
    stats = nc.dram_tensor("stats", [P, 3 * nt], f32, kind="ExternalOutput").ap()

    ti_ar = nc.alloc_sbuf_tensor("ti_ar", [P, ARENA], f32).ap()
    tt_ar = nc.alloc_sbuf_tensor("tt_ar", [P, ARENA], f32).ap()
    # dummy compute outputs (accumulator taps are the real product);
    # two per engine, alternated so consecutive same-engine instructions
    # never write the same buffer (deep-pipeline WAW)
    sd = [nc.alloc_sbuf_tensor(f"sd{i}", [P, max_f], bf16).ap() for i in range(2)]
    sa = [nc.alloc_sbuf_tensor(f"sa{i}", [P, max_f], bf16).ap() for i in range(2)]
    st = nc.alloc_sbuf_tensor("st", [P, 3 * nt], f32).ap()

    offs_dram, offs_arena, waits = _layout(tiles)
    V = []  # cumulative vector instr count through tile t
    S = []  # cumulative scalar instr count through tile t
    v = s = 0
    for tf, mode in tiles:
        v += 2 if mode == "dve" else 1
        s += 0 if mode == "dve" else 2
        V.append(v)
        S.append(s)

    with ExitStack() as ctx:
        tile_sems = [
            ctx.enter_context(nc.semaphore(f"tile_sem{i}")) for i in range(nt)
        ]
        vec_sem = ctx.enter_context(nc.semaphore("vec_sem"))
        sc_sem = ctx.enter_context(nc.semaphore("sc_sem"))
        gp_sem = ctx.enter_context(nc.semaphore("gp_sem"))
        out_sem = ctx.enter_context(nc.semaphore("out_sem"))
        block = ctx.enter_context(nc.Block())

        @block.gpsimd
        def _(gpsimd):
            gpsimd.memset(st[:], 0.0).then_inc(gp_sem, 1)

        @block.sync
        def _(sync):
            wv = ws = 0  # highest consumer counts already waited on
            for t, (tf, mode) in enumerate(tiles):
                w = waits[t]
                if w is not None:
                    if V[w] > wv:
                        sync.wait_ge(vec_sem, V[w])
                        wv = V[w]
                    if S[w] > ws:
                        sync.wait_ge(sc_sem, S[w])
                        ws = S[w]
                o, a = offs_dram[t], offs_arena[t]
                src_i = inp[o * P : (o + tf) * P].rearrange("(p f) -> p f", p=P)
                src_t = tgt[o * P : (o + tf) * P].rearrange("(p f) -> p f", p=P)
                sem = tile_sems[t]
                sync.dma_start(out=ti_ar[:, a : a + tf], in_=src_i).then_inc(
                    sem, 16
                )
                sync.dma_start(out=tt_ar[:, a : a + tf], in_=src_t).then_inc(
                    sem, 16
                )
            # sem update on an accum instruction fires at full instruction
            # retirement (incl. the accumulator write-back), so the stats DMA
            # can depend on the compute sems directly - no fence instructions
            sync.wait_ge(vec_sem, V[-1])
            sync.wait_ge(sc_sem, S[-1])
            sync.wait_ge(gp_sem, 1)
            sync.dma_start(out=stats[:], in_=st[:]).then_inc(out_sem, 16)
            sync.wait_ge(out_sem, 16)

        @block.vector
        def _(vector):
            vector.wait_ge(gp_sem, 1)
            vi = 0
            for t, (tf, mode) in enumerate(tiles):
                a = offs_arena[t]
                vector.wait_ge(tile_sems[t], 32)
                if vi >= 2:
                    # scratch-reuse self-wait; satisfied by in-order retirement
                    vector.wait_ge(vec_sem, vi - 1)
                vector.scalar_tensor_tensor(
                    out=sd[vi % 2][:, :tf],
                    in0=ti_ar[:, a : a + tf],
                    scalar=THRESH,
                    in1=tt_ar[:, a : a + tf],
                    op0=mybir.AluOpType.is_gt,
                    op1=mybir.AluOpType.mult,
                    accum_out=st[:, t : t + 1],
                ).then_inc(vec_sem, 1)
                vi += 1
                if mode == "dve":
                    if vi >= 2:
                        vector.wait_ge(vec_sem, vi - 1)
                    vector.scalar_tensor_tensor(
                        out=sd[vi % 2][:, :tf],
                        in0=ti_ar[:, a : a + tf],
                        scalar=THRESH,
                        in1=tt_ar[:, a : a + tf],
                        op0=mybir.AluOpType.is_gt,
                        op1=mybir.AluOpType.add,
                        accum_out=st[:, nt + t : nt + t + 1],
                    ).then_inc(vec_sem, 1)
                    vi += 1

        @block.scalar
        def _(scalar):
            scalar.wait_ge(gp_sem, 1)
            si = 0
            for t, (tf, mode) in enumerate(tiles):
                if mode == "dve":
                    continue
                a = offs_arena[t]
                scalar.wait_ge(tile_sems[t], 32)
                if si >= 2:
                    scalar.wait_ge(sc_sem, si - 1)
                scalar.activation(
                    out=sa[0][:, :tf],
                    in_=tt_ar[:, a : a + tf],
                    func=mybir.ActivationFunctionType.Copy,
                    accum_out=st[:, nt + t : nt + t + 1],
                ).then_inc(sc_sem, 1)
                si += 1
                if si >= 2:
                    scalar.wait_ge(sc_sem, si - 1)
                # Sign(1 - 2x) = -Sign(x - 0.5); bias=1.0 has a pre-registered
                # const AP, the host negates
                scalar.activation(
                    out=sa[1][:, :tf],
                    in_=ti_ar[:, a : a + tf],
                    func=mybir.ActivationFunctionType.Sign,
                    bias=1.0,
                    scale=-2.0,
                    accum_out=st[:, 2 * nt + t : 2 * nt + t + 1],
                ).then_inc(sc_sem, 1)
                si += 1

    nc.compile()
    return nc


def _get_nc():
    key = (TILES, N_CORES)
    if key not in _CACHE:
        _CACHE[key] = _build(*key)
    return _CACHE[key]


def kernel(input: np.ndarray, target: np.ndarray, **run_kwargs):
    inp = np.asarray(input, dtype=np.float32).reshape(N_CORES, PER_CORE)
    tgt = np.asarray(target, dtype=np.float32).reshape(N_CORES, PER_CORE)

    nc = _get_nc()
    in_maps = [
        {"input": np.ascontiguousarray(inp[c]), "target": np.ascontiguousarray(tgt[c])}
        for c in range(N_CORES)
    ]
    res = run_bass_kernel_spmd(nc, in_maps, core_ids=list(range(N_CORES)), **run_kwargs)

    nt = len(TILES)
    act_tiles = [t for t, (_, m) in enumerate(TILES) if m == "act"]
    inter = 0.0
    loss2 = 0.0
    sign_sum = 0.0
    for c in range(N_CORES):
        stats = res.results[c]["stats"].astype(np.float64)
        inter += stats[:, :nt].sum()
        # "dve" tiles: direct (bin + tgt) partials; "act" tiles: Copy -> tgt sums
        loss2 += stats[:, nt : 2 * nt].sum()
        sign_sum += sum(stats[:, 2 * nt + t].sum() for t in act_tiles)
    # "act" tiles' bin count from sign sums: S' = #lt - #gt -> bin = (n - S')/2
    n_act_elems = N_CORES * P * sum(TILES[t][0] for t in act_tiles)
    loss2 += (n_act_elems - sign_sum) / 2.0

    loss1 = np.float32(2.0 * inter)
    loss2 = np.float32(loss2)
    out = (loss1, loss2)
    if run_kwargs.get("trace"):
        return out, res
    return out


# revision 5
# speedup vs baseline: 1.3860x; 1.0106x over previous
"""DiceLoss partial-sum kernel for Trainium2 (8 NeuronCores, data-parallel).

Computes, for input/target of shape (32, 1, 1024, 1024) fp32:
    bin   = (input > 0.5) ? 1.0 : 0.0
    loss1 = 2 * sum(bin * target)
    loss2 = sum(bin) + sum(target)
and returns (loss1, loss2) as fp32 scalars (same structure as the reference).

Sharding: batch dim N=32 is split 4-per-core across 8 cores. Each core
streams its 16 MiB input + 16 MiB target shard through SBUF as [128, F]
fp32 tiles via HWDGE DMA. The problem is HBM-bound (~80 us of DMA per core
at the ~425 GB/s fair share of the chip's aggregate bandwidth; strided
high-byte reads don't help - the DMA ISA lowers non-contiguous innermost
dims to per-element descriptors).

v2 design (deep-runway): the previous 4-slot ring paced late-tile DMAs on
compute completion, which on contended runs collapsed the last ~2 MiB into
a 20+ us latency-bound convoy (queues drained to 1-2 small tiles in
flight). Now each tensor streams through a 20480-elem/partition SBUF arena
(20 MiB total runway): the first 5 x 4096-wide tile pairs are DMA'd with
no waits at all, later tiles only wait on consumers of the early tiles
whose arena range they reuse (satisfied long before the queues drain), so
the DMA queues stay descriptor-fed for the whole 32 MiB and the stream
runs at the HBM share until the last byte. The tile sizes taper
(...4096, 2048, 1024, 512, 512) so the compute tail after the last byte
lands is ~1 us.

Engines: per tile, vector does STT (in>0.5)*tgt accum -> loss1 column;
"dve" tiles also STT (in>0.5)+tgt accum -> loss2 column (exact). "act"
tiles instead use scalar: Copy(tgt) accum -> tgt column and Sign(1-2*in)
accum -> sign column (bin count recovered on host as (n - S')/2). Split
balances vector/scalar at ~44 us each, both well under the ~80 us DMA
window. Dummy STT/ACT outputs are written to small bf16 scratch (the
accumulator taps are what we keep), freeing SBUF for the arena. gpsimd
zeroes the stats tile once at start. Per-partition partial sums land in a
[128, 3*nt] stats tile DMA'd out per core; the final tiny reduction over
cores/partitions/tiles happens on the host in float64.
"""

from contextlib import ExitStack

import numpy as np

try:
    import concourse.bass  # noqa: F401
except ImportError:  # pragma: no cover - path fallback for bare containers
    import sys

    for _p in ("/opt/trn_rl_repo", "/root/.axon_site/_ro/trn_rl_repo"):
        if _p not in sys.path:
            sys.path.insert(0, _p)

import concourse.bacc as bacc
import concourse.mybir as mybir
from concourse.bass_utils import run_bass_kernel_spmd

N_CORES = 8
FULL_SHAPE = (32, 1, 1024, 1024)
FULL_ELEMS = 32 * 1024 * 1024
PER_CORE = FULL_ELEMS // N_CORES  # 4_194_304
P = 128
FREE = PER_CORE // P  # 32768 fp32 elements per partition per tensor
THRESH = 0.5
ARENA = 20480  # fp32 elems/partition per tensor arena (80 KiB); 5 x 4096

# (free_size, loss2_mode); sizes sum to FREE. Tiles are placed at
# cumulative-offset mod ARENA; no tile straddles the wrap. The taper +
# mode order came from brute-forcing a two-engine in-order timing model
# against the measured 428 GB/s arrival rate: scalar handles loss2 for
# the big early tiles (its 2-pass chain must not extend past the last
# byte), the two "dve" tiles relieve scalar right where its chain would
# otherwise back up, and the final tiny act tiles cost ~0.7 us.
TILES = tuple(
    [(4096, "act"), (4096, "act"), (4096, "act"), (4096, "act"),
     (4096, "act"), (4096, "act"),
     (2048, "act"), (2048, "dve"), (2048, "act"),
     (1024, "dve"), (512, "act"), (256, "dve"), (256, "dve")]
)
assert sum(f for f, _ in TILES) == FREE

_CACHE: dict = {}


def _layout(tiles):
    """Arena offsets + (first tile whose consumers must finish before
    this tile's DMA may overwrite its arena range, or None)."""
    offs_dram, offs_arena, waits = [], [], []
    ranges = []  # per tile: (arena_start, arena_end)
    off = 0
    for t, (tf, _) in enumerate(tiles):
        offs_dram.append(off)
        a = off % ARENA
        assert a + tf <= ARENA
        offs_arena.append(a)
        # latest earlier tile whose arena range intersects [a, a+tf)
        w = None
        for u in range(t - 1, -1, -1):
            ua, ue = ranges[u]
            if ua < a + tf and a < ue:
                w = u
                break
        waits.append(w)
        ranges.append((a, a + tf))
        off += tf
    return offs_dram, offs_arena, waits


def _build(tiles: tuple, n_cores: int):
    f32 = mybir.dt.float32
    bf16 = mybir.dt.bfloat16
    nt = len(tiles)
    per_core = P * sum(f for f, _ in tiles)
    max_f = max(f for f, _ in tiles)
    nc = bacc.Bacc(
        "TRN2", target_bir_lowering=False, debug=False, num_devices=n_cores
    )
    inp = nc.dram_tensor("input", [per_core], f32, kind="ExternalInput").ap()
    tgt = nc.dram_tensor("target", [per_core], f32, kind="ExternalInput").ap()
    stats = nc.dram_tensor("stats", [P, 3 * nt], f32, kind="ExternalOutput").ap()

    ti_ar = nc.alloc_sbuf_tensor("ti_ar", [P, ARENA], f32).ap()
    tt_ar = nc.alloc_sbuf_tensor("tt_ar", [P, ARENA], f32).ap()
    # dummy compute outputs (accumulator taps are the real product);
    # two per engine, alternated so consecutive same-engine instructions
    # never write the same buffer (deep-pipeline WAW)
    sd = [nc.alloc_sbuf_tensor(f"sd{i}", [P, max_f], bf16).ap() for i in range(2)]
    sa = [nc.alloc_sbuf_tensor(f"sa{i}", [P, max_f], bf16).ap() for i in range(2)]
    st = nc.alloc_sbuf_tensor("st", [P, 3 * nt], f32).ap()

    offs_dram, offs_arena, waits = _layout(tiles)
    V = []  # cumulative vector instr count through tile t
    S = []  # cumulative scalar instr count through tile t
    v = s = 0
    for tf, mode in tiles:
        v += 2 if mode == "dve" else 1
        s += 0 if mode == "dve" else 2
        V.append(v)
        S.append(s)

    with ExitStack() as ctx:
        tile_sems = [
            ctx.enter_context(nc.semaphore(f"tile_sem{i}")) for i in range(nt)
        ]
        vec_sem = ctx.enter_context(nc.semaphore("vec_sem"))
        sc_sem = ctx.enter_context(nc.semaphore("sc_sem"))
        gp_sem = ctx.enter_context(nc.semaphore("gp_sem"))
        out_sem = ctx.enter_context(nc.semaphore("out_sem"))
        block = ctx.enter_context(nc.Block())

        @block.gpsimd
        def _(gpsimd):
            gpsimd.memset(st[:], 0.0).then_inc(gp_sem, 1)

        @block.sync
        def _(sync):
            wv = ws = 0  # highest consumer counts already waited on
            for t, (tf, mode) in enumerate(tiles):
                w = waits[t]
                if w is not None:
                    if V[w] > wv:
                        sync.wait_ge(vec_sem, V[w])
                        wv = V[w]
                    if S[w] > ws:
                        sync.wait_ge(sc_sem, S[w])
                        ws = S[w]
                o, a = offs_dram[t], offs_arena[t]
                src_i = inp[o * P : (o + tf) * P].rearrange("(p f) -> p f", p=P)
                src_t = tgt[o * P : (o + tf) * P].rearrange("(p f) -> p f", p=P)
                sem = tile_sems[t]
                sync.dma_start(out=ti_ar[:, a : a + tf], in_=src_i).then_inc(
                    sem, 16
                )
                sync.dma_start(out=tt_ar[:, a : a + tf], in_=src_t).then_inc(
                    sem, 16
                )
            # sem update on an accum instruction fires at full instruction
            # retirement (incl. the accumulator write-back), so the stats DMA
            # can depend on the compute sems directly - no fence instructions
            sync.wait_ge(vec_sem, V[-1])
            sync.wait_ge(sc_sem, S[-1])
            sync.wait_ge(gp_sem, 1)
            sync.dma_start(out=stats[:], in_=st[:]).then_inc(out_sem, 16)
            sync.wait_ge(out_sem, 16)

        @block.vector
        def _(vector):
            vector.wait_ge(gp_sem, 1)
            vi = 0
            for t, (tf, mode) in enumerate(tiles):
                a = offs_arena[t]
                vector.wait_ge(tile_sems[t], 32)
                if vi >= 2:
                    # scratch-reuse self-wait; satisfied by in-order retirement
                    vector.wait_ge(vec_sem, vi - 1)
                vector.scalar_tensor_tensor(
                    out=sd[vi % 2][:, :tf],
                    in0=ti_ar[:, a : a + tf],
                    scalar=THRESH,
                    in1=tt_ar[:, a : a + tf],
                    op0=mybir.AluOpType.is_gt,
                    op1=mybir.AluOpType.mult,
                    accum_out=st[:, t : t + 1],
                ).then_inc(vec_sem, 1)
                vi += 1
                if mode == "dve":
                    if vi >= 2:
                        vector.wait_ge(vec_sem, vi - 1)
                    vector.scalar_tensor_tensor(
                        out=sd[vi % 2][:, :tf],
                        in0=ti_ar[:, a : a + tf],
                        scalar=THRESH,
                        in1=tt_ar[:, a : a + tf],
                        op0=mybir.AluOpType.is_gt,
                        op1=mybir.AluOpType.add,
                        accum_out=st[:, nt + t : nt + t + 1],
                    ).then_inc(vec_sem, 1)
                    vi += 1

        @block.scalar
        def _(scalar):
            scalar.wait_ge(gp_sem, 1)
            si = 0
            for t, (tf, mode) in enumerate(tiles):
                if mode == "dve":
                    continue
                a = offs_arena[t]
                scalar.wait_ge(tile_sems[t], 32)
                if si >= 2:
                    scalar.wait_ge(sc_sem, si - 1)
                scalar.activation(
                    out=sa[0][:, :tf],
                    in_=tt_ar[:, a : a + tf],
                    func=mybir.ActivationFunctionType.Copy,
                    accum_out=st[:, nt + t : nt + t + 1],
                ).then_inc(sc_sem, 1)
                si += 1
                if si >= 2:
                    scalar.wait_ge(sc_sem, si - 1)
                # Sign(1 - 2x) = -Sign(x - 0.5); bias=1.0 has a pre-registered
                # const AP, the host negates
                scalar.activation(
                    out=sa[1][:, :tf],
                    in_=ti_ar[:, a : a + tf],
                    func=mybir.ActivationFunctionType.Sign,
                    bias=1.0,
                    scale=-2.0,
                    accum_out=st[:, 2 * nt + t : 2 * nt + t + 1],
                ).then_inc(sc_sem, 1)
                si += 1

    nc.compile()
    return nc


def _get_nc():
    key = (TILES, N_CORES)
    if key not in _CACHE:
        _CACHE[key] = _build(*key)
    return _CACHE[key]


def kernel(input: np.ndarray, target: np.ndarray, **run_kwargs):
    inp = np.asarray(input, dtype=np.float32).reshape(N_CORES, PER_CORE)
    tgt = np.asarray(target, dtype=np.float32).reshape(N_CORES, PER_CORE)

    nc = _get_nc()
    in_maps = [
        {"input": np.ascontiguousarray(inp[c]), "target": np.ascontiguousarray(tgt[c])}
        for c in range(N_CORES)
    ]
    res = run_bass_kernel_spmd(nc, in_maps, core_ids=list(range(N_CORES)), **run_kwargs)

    nt = len(TILES)
    act_tiles = [t for t, (_, m) in enumerate(TILES) if m == "act"]
    inter = 0.0
    loss2 = 0.0
    sign_sum = 0.0
    for c in range(N_CORES):
        stats = res.results[c]["stats"].astype(np.float64)
        inter += stats[:, :nt].sum()
        # "dve" tiles: direct (bin + tgt) partials; "act" tiles: Copy -> tgt sums
        loss2 += stats[:, nt : 2 * nt].sum()
        sign_sum += sum(stats[:, 2 * nt + t].sum() for t in act_tiles)
    # "act" tiles' bin count from sign sums: S' = #lt - #gt -> bin = (n - S')/2
    n_act_elems = N_CORES * P * sum(TILES[t][0] for t in act_tiles)
    loss2 += (n_act_elems - sign_sum) / 2.0

    loss1 = np.float32(2.0 * inter)
    loss2 = np.float32(loss2)
    out = (loss1, loss2)
    if run_kwargs.get("trace"):
        return out, res
    return out


# revision 7
# speedup vs baseline: 1.3871x; 1.0008x over previous
"""DiceLoss partial-sum kernel for Trainium2 (8 NeuronCores, data-parallel).

Computes, for input/target of shape (32, 1, 1024, 1024) fp32:
    bin   = (input > 0.5) ? 1.0 : 0.0
    loss1 = 2 * sum(bin * target)
    loss2 = sum(bin) + sum(target)
and returns (loss1, loss2) as fp32 scalars (same structure as the reference).

Sharding: batch dim N=32 is split 4-per-core across 8 cores. Each core
streams its 16 MiB input + 16 MiB target shard through SBUF as [128, F]
fp32 tiles via HWDGE DMA. The problem is HBM-bound (~80 us of DMA per core
at the ~425 GB/s fair share of the chip's aggregate bandwidth; strided
high-byte reads don't help - the DMA ISA lowers non-contiguous innermost
dims to per-element descriptors).

v2 design (deep-runway): the previous 4-slot ring paced late-tile DMAs on
compute completion, which on contended runs collapsed the last ~2 MiB into
a 20+ us latency-bound convoy (queues drained to 1-2 small tiles in
flight). Now each tensor streams through a 20480-elem/partition SBUF arena
(20 MiB total runway): the first 5 x 4096-wide tile pairs are DMA'd with
no waits at all, later tiles only wait on consumers of the early tiles
whose arena range they reuse (satisfied long before the queues drain), so
the DMA queues stay descriptor-fed for the whole 32 MiB and the stream
runs at the HBM share until the last byte. The tile sizes taper
(...4096, 2048, 1024, 512, 512) so the compute tail after the last byte
lands is ~1 us.

Engines: per tile, vector does STT (in>0.5)*tgt accum -> loss1 column;
"dve" tiles also STT (in>0.5)+tgt accum -> loss2 column (exact). "act"
tiles instead use scalar: Copy(tgt) accum -> tgt column and Sign(1-2*in)
accum -> sign column (bin count recovered on host as (n - S')/2). Split
balances vector/scalar at ~44 us each, both well under the ~80 us DMA
window. Dummy STT/ACT outputs are written to small bf16 scratch (the
accumulator taps are what we keep), freeing SBUF for the arena. gpsimd
zeroes the stats tile once at start. Per-partition partial sums land in a
[128, 3*nt] stats tile DMA'd out per core; the final tiny reduction over
cores/partitions/tiles happens on the host in float64.
"""

from contextlib import ExitStack

import numpy as np

try:
    import concourse.bass  # noqa: F401
except ImportError:  # pragma: no cover - path fallback for bare containers
    import sys

    for _p in ("/opt/trn_rl_repo", "/root/.axon_site/_ro/trn_rl_repo"):
        if _p not in sys.path:
            sys.path.insert(0, _p)

import concourse.bacc as bacc
import concourse.mybir as mybir
from concourse.bass_utils import run_bass_kernel_spmd

N_CORES = 8
FULL_SHAPE = (32, 1, 1024, 1024)
FULL_ELEMS = 32 * 1024 * 1024
PER_CORE = FULL_ELEMS // N_CORES  # 4_194_304
P = 128
FREE = PER_CORE // P  # 32768 fp32 elements per partition per tensor
THRESH = 0.5
ARENA = 20480  # fp32 elems/partition per tensor arena (80 KiB); 5 x 4096

# (free_size, loss2_mode); sizes sum to FREE. Tiles are placed at
# cumulative-offset mod ARENA; no tile straddles the wrap. The taper +
# mode order came from brute-forcing a two-engine in-order timing model
# against the measured 428 GB/s arrival rate: scalar handles loss2 for
# the big early tiles (its 2-pass chain must not extend past the last
# byte), the two "dve" tiles relieve scalar right where its chain would
# otherwise back up, and the final tiny act tiles cost ~0.7 us.
TILES = tuple(
    [(4096, "act"), (4096, "act"), (4096, "act"), (4096, "act"),
     (4096, "act"), (4096, "act"),
     (2048, "act"), (2048, "dve"), (2048, "act"),
     (1024, "dve"), (512, "act"), (256, "dve"), (256, "dve")]
)
assert sum(f for f, _ in TILES) == FREE

_CACHE: dict = {}


def _layout(tiles):
    """Arena offsets + (first tile whose consumers must finish before
    this tile's DMA may overwrite its arena range, or None)."""
    offs_dram, offs_arena, waits = [], [], []
    ranges = []  # per tile: (arena_start, arena_end)
    off = 0
    for t, (tf, _) in enumerate(tiles):
        offs_dram.append(off)
        a = off % ARENA
        assert a + tf <= ARENA
        offs_arena.append(a)
        # latest earlier tile whose arena range intersects [a, a+tf)
        w = None
        for u in range(t - 1, -1, -1):
            ua, ue = ranges[u]
            if ua < a + tf and a < ue:
                w = u
                break
        waits.append(w)
        ranges.append((a, a + tf))
        off += tf
    return offs_dram, offs_arena, waits


def _build(tiles: tuple, n_cores: int):
    f32 = mybir.dt.float32
    bf16 = mybir.dt.bfloat16
    nt = len(tiles)
    per_core = P * sum(f for f, _ in tiles)
    max_f = max(f for f, _ in tiles)
    nc = bacc.Bacc(
        "TRN2", target_bir_lowering=False, debug=False, num_devices=n_cores
    )
    inp = nc.dram_tensor("input", [per_core], f32, kind="ExternalInput").ap()
    tgt = nc.dram_tensor("target", [per_core], f32, kind="ExternalInput").ap()
    stats = nc.dram_tensor("stats", [P, 3 * nt], f32, kind="ExternalOutput").ap()

    ti_ar = nc.alloc_sbuf_tensor("ti_ar", [P, ARENA], f32).ap()
    tt_ar = nc.alloc_sbuf_tensor("tt_ar", [P, ARENA], f32).ap()
    # dummy compute outputs (accumulator taps are the real product);
    # two per engine, alternated so consecutive same-engine instructions
    # never write the same buffer (deep-pipeline WAW)
    sd = [nc.alloc_sbuf_tensor(f"sd{i}", [P, max_f], bf16).ap() for i in range(2)]
    sa = [nc.alloc_sbuf_tensor(f"sa{i}", [P, max_f], bf16).ap() for i in range(2)]
    st = nc.alloc_sbuf_tensor("st", [P, 3 * nt], f32).ap()

    offs_dram, offs_arena, waits = _layout(tiles)
    V = []  # cumulative vector instr count through tile t
    S = []  # cumulative scalar instr count through tile t
    v = s = 0
    for tf, mode in tiles:
        v += 2 if mode == "dve" else 1
        s += 0 if mode == "dve" else 2
        V.append(v)
        S.append(s)

    with ExitStack() as ctx:
        tile_sems = [
            ctx.enter_context(nc.semaphore(f"tile_sem{i}")) for i in range(nt)
        ]
        vec_sem = ctx.enter_context(nc.semaphore("vec_sem"))
        sc_sem = ctx.enter_context(nc.semaphore("sc_sem"))
        gp_sem = ctx.enter_context(nc.semaphore("gp_sem"))
        out_sem = ctx.enter_context(nc.semaphore("out_sem"))
        block = ctx.enter_context(nc.Block())

        @block.gpsimd
        def _(gpsimd):
            gpsimd.memset(st[:], 0.0).then_inc(gp_sem, 1)

        # the first two tiles' target loads are issued from the scalar
        # engine's HWDGE queue: during the ramp the sync queue alone is
        # issue-limited (~600 ns per dma_start before descriptors exist),
        # so kicking a second queue brings the first tile pairs in ~2x
        # sooner; past the ramp everything flows through the sync queue
        # (two active queues split the shared DMA-engine pool and slightly
        # hurt the steady rate, so only the ramp uses the second one)
        N_KICK = 2

        @block.sync
        def _(sync):
            wv = ws = 0  # highest consumer counts already waited on
            for t, (tf, mode) in enumerate(tiles):
                w = waits[t]
                if w is not None:
                    if V[w] > wv:
                        sync.wait_ge(vec_sem, V[w])
                        wv = V[w]
                    if S[w] > ws:
                        sync.wait_ge(sc_sem, S[w])
                        ws = S[w]
                o, a = offs_dram[t], offs_arena[t]
                src_i = inp[o * P : (o + tf) * P].rearrange("(p f) -> p f", p=P)
                src_t = tgt[o * P : (o + tf) * P].rearrange("(p f) -> p f", p=P)
                sem = tile_sems[t]
                sync.dma_start(out=ti_ar[:, a : a + tf], in_=src_i).then_inc(
                    sem, 16
                )
                if t >= N_KICK:
                    sync.dma_start(out=tt_ar[:, a : a + tf], in_=src_t).then_inc(
                        sem, 16
                    )
            # sem update on an accum instruction fires at full instruction
            # retirement (incl. the accumulator write-back), so the stats DMA
            # can depend on the compute sems directly - no fence instructions
            sync.wait_ge(vec_sem, V[-1])
            sync.wait_ge(sc_sem, S[-1])
            sync.wait_ge(gp_sem, 1)
            sync.dma_start(out=stats[:], in_=st[:]).then_inc(out_sem, 16)
            sync.wait_ge(out_sem, 16)

        @block.vector
        def _(vector):
            vector.wait_ge(gp_sem, 1)
            vi = 0
            for t, (tf, mode) in enumerate(tiles):
                a = offs_arena[t]
                vector.wait_ge(tile_sems[t], 32)
                if vi >= 2:
                    # scratch-reuse self-wait; satisfied by in-order retirement
                    vector.wait_ge(vec_sem, vi - 1)
                vector.scalar_tensor_tensor(
                    out=sd[vi % 2][:, :tf],
                    in0=ti_ar[:, a : a + tf],
                    scalar=THRESH,
                    in1=tt_ar[:, a : a + tf],
                    op0=mybir.AluOpType.is_gt,
                    op1=mybir.AluOpType.mult,
                    accum_out=st[:, t : t + 1],
                ).then_inc(vec_sem, 1)
                vi += 1
                if mode == "dve":
                    if vi >= 2:
                        vector.wait_ge(vec_sem, vi - 1)
                    vector.scalar_tensor_tensor(
                        out=sd[vi % 2][:, :tf],
                        in0=ti_ar[:, a : a + tf],
                        scalar=THRESH,
                        in1=tt_ar[:, a : a + tf],
                        op0=mybir.AluOpType.is_gt,
                        op1=mybir.AluOpType.add,
                        accum_out=st[:, nt + t : nt + t + 1],
                    ).then_inc(vec_sem, 1)
                    vi += 1

        @block.scalar
        def _(scalar):
            for t in range(N_KICK):
                tf = tiles[t][0]
                o, a = offs_dram[t], offs_arena[t]
                src_t = tgt[o * P : (o + tf) * P].rearrange("(p f) -> p f", p=P)
                scalar.dma_start(
                    out=tt_ar[:, a : a + tf], in_=src_t
                ).then_inc(tile_sems[t], 16)
            scalar.wait_ge(gp_sem, 1)
            si = 0
            for t, (tf, mode) in enumerate(tiles):
                if mode == "dve":
                    continue
                a = offs_arena[t]
                scalar.wait_ge(tile_sems[t], 32)
                if si >= 2:
                    scalar.wait_ge(sc_sem, si - 1)
                scalar.activation(
                    out=sa[0][:, :tf],
                    in_=tt_ar[:, a : a + tf],
                    func=mybir.ActivationFunctionType.Copy,
                    accum_out=st[:, nt + t : nt + t + 1],
                ).then_inc(sc_sem, 1)
                si += 1
                if si >= 2:
                    scalar.wait_ge(sc_sem, si - 1)
                # Sign(1 - 2x) = -Sign(x - 0.5); bias=1.0 has a pre-registered
                # const AP, the host negates
                scalar.activation(
                    out=sa[1][:, :tf],
                    in_=ti_ar[:, a : a + tf],
                    func=mybir.ActivationFunctionType.Sign,
                    bias=1.0,
                    scale=-2.0,
                    accum_out=st[:, 2 * nt + t : 2 * nt + t + 1],
                ).then_inc(sc_sem, 1)
                si += 1

    nc.compile()
    return nc


def _get_nc():
    key = (TILES, N_CORES)
    if key not in _CACHE:
        _CACHE[key] = _build(*key)
    return _CACHE[key]


def kernel(input: np.ndarray, target: np.ndarray, **run_kwargs):
    inp = np.asarray(input, dtype=np.float32).reshape(N_CORES, PER_CORE)
    tgt = np.asarray(target, dtype=np.float32).reshape(N_CORES, PER_CORE)

    nc = _get_nc()
    in_maps = [
        {"input": np.ascontiguousarray(inp[c]), "target": np.ascontiguousarray(tgt[c])}
        for c in range(N_CORES)
    ]
    res = run_bass_kernel_spmd(nc, in_maps, core_ids=list(range(N_CORES)), **run_kwargs)

    nt = len(TILES)
    act_tiles = [t for t, (_, m) in enumerate(TILES) if m == "act"]
    inter = 0.0
    loss2 = 0.0
    sign_sum = 0.0
    for c in range(N_CORES):
        stats = res.results[c]["stats"].astype(np.float64)
        inter += stats[:, :nt].sum()
        # "dve" tiles: direct (bin + tgt) partials; "act" tiles: Copy -> tgt sums
        loss2 += stats[:, nt : 2 * nt].sum()
        sign_sum += sum(stats[:, 2 * nt + t].sum() for t in act_tiles)
    # "act" tiles' bin count from sign sums: S' = #lt - #gt -> bin = (n - S')/2
    n_act_elems = N_CORES * P * sum(TILES[t][0] for t in act_tiles)
    loss2 += (n_act_elems - sign_sum) / 2.0

    loss1 = np.float32(2.0 * inter)
    loss2 = np.float32(loss2)
    out = (loss1, loss2)
    if run_kwargs.get("trace"):
        return out, res
    return out
